# revision 12
# baseline (speedup 1.0000x reference)
"""Trainium2 Bass kernel for nn_NeighborAttention.

Algebraic structure: the attention query is one mean-pooled neighbor
vector per batch broadcast over the sequence, so the [seq, seq]
attention collapses to one weight vector per head and the attention
output is a single vector v per batch added to every row of x before
the LayerNorm.  k/v projections are never materialized.

Per core (data-parallel over batch): x bf16 (2 MB) + x^T fp8 pair-
layout (1 MB) + 4 weight matrices fp8 pair-layout scaled x64 (4 MB) +
bf16 output (2 MB).  All large GEMMs run in fp8 DoubleRow mode (two
128-row contraction chunks per instruction at 0.5 cyc/row).  The
softmax weights are scaled x256 via an exp bias so they sit in fp8
normal range; the scale cancels through the softmax denominator.
Residual+LayerNorm is evaluated ON THE PE as
  out_tile = diag(rstd) @ x_tile + [rstd; nmr]^T @ [v; ones]
with per-row x statistics from bn_stats during the DMA fill and the
v cross-term via one DoubleRow GEMM against x^T.  The scalar engine
runs only Exp/Sqrt/Square/Copy (resident ACT table sets).
Host-side prep is layout/dtype only (transpose, pair-interleave,
scale-by-constant, cast).
"""

import numpy as np
import ml_dtypes
from contextlib import ExitStack

try:
    import concourse.bass as bass
except ImportError:  # pragma: no cover - fallback for bare containers
    import sys
    sys.path.insert(0, "/opt/trn_rl_repo")
    import concourse.bass as bass

import concourse.tile as tile
from concourse import bacc, mybir
from concourse import bass_utils
from concourse.alu_op_type import AluOpType

F32 = mybir.dt.float32
BF16 = mybir.dt.bfloat16
FP8 = mybir.dt.float8e4
I32 = mybir.dt.int32
AF = mybir.ActivationFunctionType
AX = mybir.AxisListType
DR = mybir.MatmulPerfMode.DoubleRow

BS, SEQ, DIM, NH, DH, NNB = 8, 1024, 1024, 16, 64, 50
NT = SEQ // 128   # seq tiles
NJ = DIM // 128   # dim chunks
NP = NJ // 2      # contraction pairs
N_CORES = 8

WS = 64.0         # fp8 weight pre-scale (host)
SQB = 64.0        # q-vector scale inside blk (fp8 range)
SQK = 128.0       # qk scale (fp8 range); exp descales
LN_SW = 4.1588830833596715  # ln(64): softmax weight scale, cancels in rden
SPN = 16.0        # pooled scale (fp8 range)
SV = 64.0         # v scale for the Sxv cross-term operand

_cache = {}


def _build(flags):
    use_qb, use_kb, use_vb, use_ob, use_g, use_b, use_mask = flags
    nc = bacc.Bacc("TRN2", target_bir_lowering=False, debug=False,
                   enable_asserts=True, num_devices=N_CORES)

    def din(name, shape, dt):
        return nc.dram_tensor(name, shape, dt, kind="ExternalInput").ap()

    x_d = din("x", [SEQ, DIM], BF16)
    xtp_d = din("xtp", [128, NJ * SEQ], FP8)     # pair layout of x^T
    qwt_d = din("qwt", [128, NJ * DIM], FP8)     # pair layouts, x WS
    kw_d = din("kw", [128, NJ * DIM], FP8)
    vwt_d = din("vwt", [128, NJ * DIM], FP8)
    owt_d = din("owt", [128, NJ * DIM], FP8)
    xnb_d = din("xnb", [NNB, DIM], BF16)
    nmc_d = din("nmc", [NNB], BF16)
    nmr_d = din("nmr", [NNB], F32)
    i128_d = din("i128", [128, 128], BF16)
    qb_d = din("qb", [DIM], F32) if use_qb else None
    kbt_d = din("kbt", [128, NJ], BF16) if use_kb else None
    vb_d = din("vbt", [128, NJ], BF16) if use_vb else None  # x (SPN*WS)
    ob_d = din("ob", [DIM], F32) if use_ob else None
    g_d = din("lng", [DIM], F32) if use_g else None
    b_d = din("lnb", [DIM], F32) if use_b else None
    mask_d = din("mask", [SEQ], I32) if use_mask else None
    out_d = nc.dram_tensor("out", [SEQ, DIM], BF16, kind="ExternalOutput").ap()

    with tile.TileContext(nc) as tc, ExitStack() as ctx:
        wpool = ctx.enter_context(tc.tile_pool(name="wts", bufs=1))
        spool = ctx.enter_context(tc.tile_pool(name="small", bufs=1))
        opool = ctx.enter_context(tc.tile_pool(name="o", bufs=4))
        pwide = ctx.enter_context(tc.tile_pool(name="pw", bufs=2, space="PSUM"))
        psmall = ctx.enter_context(tc.tile_pool(name="ps", bufs=2, space="PSUM"))

        # ---------------- DMAs (issue order ~ arrival order) ----------------
        xnb_t = spool.tile([NNB, DIM], BF16, tag="xnb")
        nc.sync.dma_start(xnb_t[:], xnb_d[:])
        nmc_t = spool.tile([NNB, 1], BF16, tag="nmc")
        nc.sync.dma_start(nmc_t[:], nmc_d.unsqueeze(1))
        nmr_t = spool.tile([1, NNB], F32, tag="nmr")
        nc.sync.dma_start(nmr_t[:], nmr_d.unsqueeze(0))
        i128_t = spool.tile([128, 128], BF16, tag="i128")
        nc.sync.dma_start(i128_t[:], i128_d[:])

        def row_tile(d_ap, tag):
            t = spool.tile([1, DIM], F32, tag=tag)
            nc.sync.dma_start(t[:], d_ap.unsqueeze(0))
            return t

        qb_t = row_tile(qb_d, "qbr") if use_qb else None
        ob_t = row_tile(ob_d, "obr") if use_ob else None
        if use_vb:
            vbt_t = spool.tile([128, NJ], BF16, tag="vbt")
            nc.sync.dma_start(vbt_t[:], vb_d[:])
        g_t = row_tile(g_d, "gr") if use_g else None
        b_t = row_tile(b_d, "br") if use_b else None
        if use_kb:
            kbt_t = spool.tile([128, NJ], BF16, tag="kbt")
            nc.sync.dma_start(kbt_t[:], kbt_d[:])
        if use_mask:
            mrow_t = spool.tile([1, SEQ], I32, tag="mrow")
            nc.sync.dma_start(mrow_t[:], mask_d.unsqueeze(0))

        def load_pairs(d_ap, tagp, cols):
            """[128, NJ*cols] fp8 pair-layout matrix, two half-DMAs.
            Returns 4D view [128, NP, 2, cols]."""
            t = wpool.tile([128, NJ * cols], FP8, tag=tagp)
            half = (NJ * cols) // 2
            nc.sync.dma_start(t[:, 0:half], d_ap[:, 0:half])
            nc.sync.dma_start(t[:, half:], d_ap[:, half:])
            return t[:].rearrange("p (P i c) -> p P i c", P=NP, i=2)

        qwtp = load_pairs(qwt_d, "qwt", DIM)
        kwp = load_pairs(kw_d, "kw", DIM)
        xtp = load_pairs(xtp_d, "xt", SEQ)

        # x bf16 (residual + pooled source), as one [128, NT*DIM] tile
        xbig = wpool.tile([128, NT * DIM], BF16, tag="x")
        xv_d = x_d.rearrange("(t p) c -> p t c", p=128)
        xv = xbig[:].rearrange("p (t c) -> p t c", t=NT)
        nc.sync.dma_start(xv[:, 0:NT // 2, :], xv_d[:, 0:NT // 2, :])
        nc.sync.dma_start(xv[:, NT // 2:, :], xv_d[:, NT // 2:, :])
        x_t = [xbig[:, t * DIM:(t + 1) * DIM] for t in range(NT)]

        vwtp = load_pairs(vwt_d, "vwt", DIM)
        owtp = load_pairs(owt_d, "owt", DIM)

        ones11 = spool.tile([1, 1], BF16, tag="ones11")
        nc.vector.memset(ones11[:], 1.0)
        ones1x128 = spool.tile([1, 128], F32, tag="ones1x128")
        nc.vector.memset(ones1x128[:], 1.0)
        lnsw_t = spool.tile([NH, 1], F32, tag="lnsw")
        nc.vector.memset(lnsw_t[:], LN_SW)

        # Load the two ACT table sets (exp_and_others, sqrt_and_others)
        # during the DMA fill; only Exp/Sqrt/Square/Copy are used after.
        dummy_t = spool.tile([1, 1], F32, tag="dummy")
        nc.vector.memset(dummy_t[:], 1.0)
        nc.scalar.activation(dummy_t[:], dummy_t[:], AF.Exp)
        nc.scalar.activation(dummy_t[:], dummy_t[:], AF.Sqrt)

        # fp8 copy of x for the DoubleRow pooled GEMM (gpsimd, fill window)
        xf8 = wpool.tile([128, NT * DIM], FP8, tag="xf8")
        for t in range(NT):
            nc.gpsimd.tensor_copy(xf8[:, t * DIM:(t + 1) * DIM], x_t[t])
        xf8v = xf8[:].rearrange("p (P i c) -> p P i c", P=NT // 2, i=2)

        # per-row bn stats of x during the fill (for the LN fixup)
        mvx_t = spool.tile([128, 2 * NT], F32, tag="mvx")
        mvxv = mvx_t[:].rearrange("p (t a) -> p t a", a=2)
        for t in range(NT):
            hv = x_t[t].rearrange("p (g f) -> p g f", g=2)
            st_t = spool.tile([128, 2, 6], F32, tag="st")
            nc.vector.bn_stats(st_t[:, 0, :], hv[:, 0, :])
            nc.vector.bn_stats(st_t[:, 1, :], hv[:, 1, :])
            nc.vector.bn_aggr(mvxv[:, t, :], st_t[:])

        def bcast_row(row_ap, out_tile, nrows):
            """out[p, :] = row[0, :] for p in range(nrows), via PE rank-1."""
            n = out_tile.shape[-1]
            pb = pwide.tile([128, DIM], F32, tag="wide")
            for h0 in range(0, n, 512):
                hi = min(h0 + 512, n)
                nc.tensor.matmul(pb[:nrows, h0:hi], lhsT=ones1x128[0:1, 0:nrows],
                                 rhs=row_ap[0:1, h0:hi], start=True, stop=True)
            nc.vector.tensor_copy(out_tile[:nrows, :], pb[:nrows, 0:n])

        def drmm(psum_ap, lhs4, rhs4, np_pairs):
            """DoubleRow GEMM: accumulate over np_pairs pair-chunks, two
            512-col halves.  lhs4: [128, np_pairs, 2, M] fp8 pair view."""
            for p in range(np_pairs):
                for h0 in (0, 512):
                    nc.tensor.matmul(psum_ap[:, h0:h0 + 512],
                                     lhsT=lhs4[:, p, :, :],
                                     rhs=rhs4[:, p, :, h0:h0 + 512],
                                     start=(p == 0), stop=(p == np_pairs - 1),
                                     perf_mode=DR)

        # zero-padded fp8 thin-operand pair tiles (M=16, col 0 real)
        def padded16(tag):
            t = spool.tile([128, NJ * 16], FP8, tag=tag)
            nc.vector.memset(t[:], 0.0)
            return t, t[:].rearrange("p (P i m) -> p P i m", P=NP, i=2)

        # ---------------- neighbor pooling: SxnT -> fp8 pad tile -----------
        ps8w = pwide.tile([128, DIM], F32, tag="wide")
        ps8 = ps8w[:, 0:16]
        for j in range(NJ):
            nc.tensor.matmul(ps8[:, j:j + 1], lhsT=xnb_t[:, j * 128:(j + 1) * 128],
                             rhs=nmc_t[:], start=True, stop=True)
        sxnt_t, sxntv = padded16("sxnt")
        sxcols = sxnt_t[:].rearrange("p (c m) -> p c m", m=16)[:, :, 0]
        nc.vector.tensor_copy(sxcols[:, 0:NJ], ps8[:, 0:NJ])
        cnt_t = spool.tile([1, 1], F32, tag="cnt")
        nc.vector.reduce_sum(cnt_t[:], nmr_t[:], AX.X)
        rcnt_t = spool.tile([1, 1], F32, tag="rcnt")
        nc.vector.reciprocal(rcnt_t[:], cnt_t[:])

        # ------- qvec = (qw @ xn + qb) / 8;  pqv is x WS -------------------
        pqv = pwide.tile([128, DIM], F32, tag="wide")
        drmm(pqv[0:16, :], sxntv, qwtp, NP)
        qvec_t = spool.tile([1, DIM], F32, tag="qvec")
        nc.vector.tensor_scalar(qvec_t[:], pqv[0:1, :], rcnt_t[:], 0.125 / WS,
                                AluOpType.mult, AluOpType.mult)
        if use_qb:
            qb8_t = spool.tile([1, DIM], F32, tag="qb8")
            nc.vector.tensor_scalar_mul(qb8_t[:], qb_t[:], 0.125)
            nc.vector.tensor_tensor(qvec_t[:], qvec_t[:], qb8_t[:], op=AluOpType.add)

        # ------- head-blocked qvec, fp8 pair tiles (x SQB) -----------------
        qvr_t = spool.tile([1, DIM], BF16, tag="rowb")
        nc.vector.tensor_scalar_mul(qvr_t[:], qvec_t[:], SQB)
        blkp_t = []
        for P in range(NP):
            bt = spool.tile([128, 32], FP8, tag=f"blk{P}")
            nc.vector.memset(bt[:], 0.0)
            for i in (0, 1):
                j = 2 * P + i
                pt = psmall.tile([128, 16], BF16, tag="psmb")
                nc.tensor.transpose(pt[:, 0:1], qvr_t[0:1, j * 128:(j + 1) * 128],
                                    ones11[:])
                nc.vector.tensor_copy(bt[0:64, 16 * i + 2 * j:16 * i + 2 * j + 1],
                                      pt[0:64, 0:1])
                nc.vector.tensor_copy(bt[64:128, 16 * i + 2 * j + 1:16 * i + 2 * j + 2],
                                      pt[64:128, 0:1])
            blkp_t.append(bt[:].rearrange("p (i m) -> p i m", i=2))

        # ------- qk[h, c] (x SQB*WS in psum -> qk_t x SQK) -----------------
        pqk = pwide.tile([128, DIM], F32, tag="wide")
        for P in range(NP):
            for h0 in (0, 512):
                nc.tensor.matmul(pqk[0:NH, h0:h0 + 512], lhsT=blkp_t[P],
                                 rhs=kwp[:, P, :, h0:h0 + 512],
                                 start=(P == 0), stop=(P == NP - 1),
                                 perf_mode=DR)
        qk_t = spool.tile([NH, DIM], BF16, tag="qk")
        nc.vector.tensor_scalar_mul(qk_t[:], pqk[0:NH, :], SQK / (SQB * WS))
        if use_kb:
            pqkbw = pwide.tile([128, DIM], F32, tag="wide")
            pqkb = pqkbw[:, 0:16]
            for P in range(NP):
                for i in (0, 1):
                    j = 2 * P + i
                    nc.tensor.matmul(pqkb[0:NH, 0:1],
                                     lhsT=blkp_t[P][:, i, :],
                                     rhs=kbt_t[:, j:j + 1],
                                     start=(j == 0), stop=(j == NJ - 1))
            qkb_t = spool.tile([NH, 1], F32, tag="qkb")
            # kb path: pqkb is x SQB; exp bias wants true qkb / SQK ... the
            # exp computes exp(psc/SQK + bias), so bias = qkb (true scale).
            nc.vector.tensor_scalar_mul(qkb_t[:], pqkb[0:NH, 0:1], 1.0 / SQB)

        # ------- scoresT [NH, SEQ] = qk @ x^T (x SQK) ----------------------
        qktp_t = []
        for P in range(NP):
            t = spool.tile([128, 32], FP8, tag=f"qkt{P}")
            for i in (0, 1):
                j = 2 * P + i
                pt = psmall.tile([128, 16], BF16, tag="psmb")
                nc.tensor.transpose(pt[:], qk_t[:, j * 128:(j + 1) * 128],
                                    i128_t[0:NH, 0:NH])
                nc.vector.tensor_copy(t[:, 16 * i:16 * i + 16], pt[:])
            qktp_t.append(t[:].rearrange("p (i m) -> p i m", i=2))
        psc = pwide.tile([128, DIM], F32, tag="wide")
        for P in range(NP):
            for h0 in (0, 512):
                nc.tensor.matmul(psc[0:NH, h0:h0 + 512], lhsT=qktp_t[P],
                                 rhs=xtp[:, P, :, h0:h0 + 512],
                                 start=(P == 0), stop=(P == NP - 1),
                                 perf_mode=DR)

        # ------- softmax over keys; w is x 256 via exp bias (cancels) ------
        w_t = spool.tile([NH, SEQ], BF16, tag="w")
        den_t = spool.tile([NH, 1], F32, tag="den")
        if use_kb:
            expb_t = spool.tile([NH, 1], F32, tag="expb")
            nc.vector.tensor_scalar(expb_t[:], qkb_t[:], LN_SW, None,
                                    AluOpType.add)
            expbias = expb_t[:]
        else:
            expbias = lnsw_t[:]
        if not use_mask:
            nc.scalar.activation(w_t[:], psc[0:NH, :], AF.Exp, bias=expbias,
                                 scale=1.0 / SQK, accum_out=den_t[:])
        else:
            nc.scalar.activation(w_t[:], psc[0:NH, :], AF.Exp, bias=expbias,
                                 scale=1.0 / SQK)
            mrowf_t = spool.tile([1, SEQ], F32, tag="mrowf")
            nc.vector.tensor_copy(mrowf_t[:], mrow_t[:])
            ind_t = spool.tile([1, SEQ], F32, tag="ind")
            nc.vector.tensor_scalar(ind_t[:], mrowf_t[:], 0.0, None,
                                    AluOpType.not_equal)
            m16_t = spool.tile([NH, SEQ], F32, tag="m16")
            bcast_row(ind_t, m16_t, NH)
            nc.vector.scalar_tensor_tensor(w_t[:], w_t[:], 1.0, m16_t[:],
                                           AluOpType.mult, AluOpType.mult,
                                           accum_out=den_t[:])
        rden_t = spool.tile([NH, 1], F32, tag="rden")
        nc.vector.reciprocal(rden_t[:], den_t[:])

        # ------- pooled[h, c] = sum_s w x (w x256 cancels via rden) --------
        wtp_t = []
        for P in range(NT // 2):
            t = spool.tile([128, 32], FP8, tag=f"wt{P}")
            for i in (0, 1):
                j = 2 * P + i
                pt = psmall.tile([128, 16], BF16, tag="psmb")
                nc.tensor.transpose(pt[:], w_t[:, j * 128:(j + 1) * 128],
                                    i128_t[0:NH, 0:NH])
                nc.vector.tensor_copy(t[:, 16 * i:16 * i + 16], pt[:])
            wtp_t.append(t[:].rearrange("p (i m) -> p i m", i=2))
        ppl = pwide.tile([128, DIM], F32, tag="wide")
        for P in range(NT // 2):
            for h0 in (0, 512):
                nc.tensor.matmul(ppl[0:NH, h0:h0 + 512], lhsT=wtp_t[P],
                                 rhs=xf8v[:, P, :, h0:h0 + 512],
                                 start=(P == 0), stop=(P == NT // 2 - 1),
                                 perf_mode=DR)
        pn_t = spool.tile([NH, DIM], BF16, tag="pn")
        nc.vector.tensor_scalar(pn_t[:], ppl[0:NH, :], rden_t[:], SPN,
                                AluOpType.mult, AluOpType.mult)

        # ------- context diag blocks (psum x SPN*WS) -----------------------
        pntp_t = []
        for P in range(NP):
            t = spool.tile([128, 32], FP8, tag=f"pnt{P}")
            for i in (0, 1):
                j = 2 * P + i
                pt = psmall.tile([128, 16], BF16, tag="psmb")
                nc.tensor.transpose(pt[:], pn_t[:, j * 128:(j + 1) * 128],
                                    i128_t[0:NH, 0:NH])
                nc.vector.tensor_copy(t[:, 16 * i:16 * i + 16], pt[:])
            pntp_t.append(t[:].rearrange("p (i m) -> p i m", i=2))
        pcx = pwide.tile([128, DIM], F32, tag="wide")
        for P in range(NP):
            for h0 in (0, 512):
                nc.tensor.matmul(pcx[0:NH, h0:h0 + 512], lhsT=pntp_t[P],
                                 rhs=vwtp[:, P, :, h0:h0 + 512],
                                 start=(P == 0), stop=(P == NP - 1),
                                 perf_mode=DR)
        # ctx[o] = pcx[head(o), o]: per-chunk transpose, pick halves into
        # the zero-padded fp8 pair tile (col 0 of each 16-block).
        pcs_t = spool.tile([NH, DIM], BF16, tag="pcs")
        nc.vector.tensor_copy(pcs_t[:], pcx[0:NH, :])
        cxt_t, cxtv = padded16("cxt")
        cxcols = cxt_t[:].rearrange("p (c m) -> p c m", m=16)[:, :, 0]
        for j in range(NJ):
            pt = psmall.tile([128, 16], BF16, tag="psmb")
            nc.tensor.transpose(pt[:], pcs_t[:, j * 128:(j + 1) * 128],
                                i128_t[0:NH, 0:NH])
            nc.vector.tensor_copy(cxcols[0:64, j:j + 1], pt[0:64, 2 * j:2 * j + 1])
            nc.vector.tensor_copy(cxcols[64:128, j:j + 1],
                                  pt[64:128, 2 * j + 1:2 * j + 2])
        if use_vb:
            nc.vector.tensor_tensor(cxcols[:, 0:NJ], cxcols[:, 0:NJ], vbt_t[:],
                                    op=AluOpType.add)

        # ------- out_vec = ow @ ctx + ob (psum x SPN*WS*WS) ----------------
        pov = pwide.tile([128, DIM], F32, tag="wide")
        drmm(pov[0:16, :], cxtv, owtp, NP)
        bvec_t = spool.tile([1, DIM], F32, tag="bvec")
        nc.vector.tensor_scalar_mul(bvec_t[:], pov[0:1, :], 1.0 / (SPN * WS * WS))
        if use_ob:
            nc.vector.tensor_tensor(bvec_t[:], bvec_t[:], ob_t[:], op=AluOpType.add)

        # =================== residual + LayerNorm ==========================
        # stats: mu_h = mu_x + mu_v ; var_h = var_x + var_v
        #        + 2*(Sxv/D - mu_x*mu_v) with Sxv = x @ v via PE on x^T.
        # output: out = diag(rstd) @ x + [rstd; nmr]^T @ [v; ones]  (PE)

        # v stats
        sv_t = spool.tile([1, 2], F32, tag="sv")
        nc.vector.reduce_sum(sv_t[0:1, 0:1], bvec_t[:], AX.X)
        junk_t = spool.tile([1, DIM], F32, tag="junk")
        nc.scalar.activation(junk_t[:], bvec_t[:], AF.Square,
                             accum_out=sv_t[0:1, 1:2])
        nc.vector.tensor_scalar_mul(sv_t[:], sv_t[:], 1.0 / DIM)
        muv2_t = spool.tile([1, 1], F32, tag="muv2")
        nc.vector.tensor_tensor(muv2_t[:], sv_t[0:1, 0:1], sv_t[0:1, 0:1],
                                op=AluOpType.mult)
        nc.vector.tensor_tensor(sv_t[0:1, 1:2], sv_t[0:1, 1:2], muv2_t[:],
                                op=AluOpType.subtract)
        bsc_t = spool.tile([128, 2], F32, tag="bsc")
        nc.gpsimd.partition_broadcast(bsc_t[:], sv_t[:])

        # Sxv via DoubleRow on x^T: operand v x SV in fp8 pad tile
        bvr_t = spool.tile([1, DIM], BF16, tag="bvr")
        nc.vector.tensor_scalar_mul(bvr_t[:], bvec_t[:], SV)
        bvt_t, bvtv = padded16("bvt")
        bvcols = bvt_t[:].rearrange("p (c m) -> p c m", m=16)[:, :, 0]
        for j in range(NJ):
            pt = psmall.tile([128, 16], BF16, tag="psmb")
            nc.tensor.transpose(pt[:, 0:1], bvr_t[0:1, j * 128:(j + 1) * 128],
                                ones11[:])
            nc.vector.tensor_copy(bvcols[:, j:j + 1], pt[:, 0:1])
        psxv = pwide.tile([128, DIM], F32, tag="wide")
        drmm(psxv[0:16, :], bvtv, xtp, NP)
        sxvr_t = spool.tile([1, SEQ], BF16, tag="sxvr")
        nc.vector.tensor_copy(sxvr_t[:], psxv[0:1, :])
        sxvc_t = spool.tile([128, NT], F32, tag="sxvc")
        for t in range(NT):
            pt = psmall.tile([128, 16], BF16, tag="psmb")
            nc.tensor.transpose(pt[:, 0:1], sxvr_t[0:1, t * 128:(t + 1) * 128],
                                ones11[:])
            nc.vector.tensor_copy(sxvc_t[:, t:t + 1], pt[:, 0:1])

        # batched fixup on [128, NT]
        a_t = spool.tile([128, NT], F32, tag="fa")
        nc.vector.tensor_scalar_mul(a_t[:], sxvc_t[:], 2.0 / (SV * DIM))
        b2_t = spool.tile([128, NT], F32, tag="fb")
        nc.vector.tensor_scalar_mul(b2_t[:], mvxv[:, :, 0], bsc_t[:, 0:1])
        c_t = spool.tile([128, NT], F32, tag="fc")
        nc.vector.scalar_tensor_tensor(c_t[:], b2_t[:], -2.0, a_t[:],
                                       AluOpType.mult, AluOpType.add)
        d_t = spool.tile([128, NT], F32, tag="fd")
        nc.vector.tensor_scalar(d_t[:], c_t[:], bsc_t[:, 1:2], None,
                                AluOpType.add)
        e_t = spool.tile([128, NT], F32, tag="fe")
        nc.vector.tensor_tensor(e_t[:], d_t[:], mvxv[:, :, 1], op=AluOpType.add)
        srt_t = spool.tile([128, NT], F32, tag="srt")
        nc.scalar.activation(srt_t[:], e_t[:], AF.Sqrt, bias=0.0)
        rstd_t = spool.tile([128, NT], F32, tag="rstd")
        nc.vector.reciprocal(rstd_t[:], srt_t[:])
        muh_t = spool.tile([128, NT], F32, tag="muh")
        nc.vector.tensor_scalar(muh_t[:], mvxv[:, :, 0], bsc_t[:, 0:1], None,
                                AluOpType.add)
        rn_t = spool.tile([128, 2 * NT], BF16, tag="rn")
        rnv = rn_t[:].rearrange("p (t a) -> p t a", a=2)
        nc.vector.tensor_copy(rnv[:, :, 0], rstd_t[:])
        nc.vector.scalar_tensor_tensor(rnv[:, :, 1], muh_t[:], -1.0, rstd_t[:],
                                       AluOpType.mult, AluOpType.mult)

        # [v; ones] moving operand and per-tile [rstd; nmr] rows
        vones_t = spool.tile([2, DIM], BF16, tag="vones")
        nc.vector.memset(vones_t[:], 1.0)
        nc.vector.tensor_copy(vones_t[0:1, :], bvec_t[:])
        rn2_ts = []
        for t in range(NT):
            pt = psmall.tile([128, 128], BF16, tag="psmc")
            nc.tensor.transpose(pt[0:2, :], rnv[:, t, :], i128_t[:, :])
            r2 = spool.tile([2, 128], BF16, tag=f"rn2_{t}")
            nc.vector.tensor_copy(r2[:], pt[0:2, :])
            rn2_ts.append(r2)

        if use_g:
            gb_t = spool.tile([128, DIM], F32, tag="gb")
            bcast_row(g_t, gb_t, 128)
        if use_b:
            bb_t = spool.tile([128, DIM], F32, tag="bb")
            bcast_row(b_t, bb_t, 128)

        # per-tile: PE affine -> psum -> copy (V/G/S round-robin) -> DMA
        engs = [nc.vector, nc.gpsimd, nc.scalar]
        for t in range(NT):
            diag_t = spool.tile([128, 128], BF16, tag="diag")
            nc.vector.tensor_scalar_mul(diag_t[:], i128_t[:], rstd_t[:, t:t + 1])
            pout = pwide.tile([128, DIM], F32, tag="wide")
            for h0 in (0, 512):
                nc.tensor.matmul(pout[:, h0:h0 + 512], lhsT=diag_t[:],
                                 rhs=x_t[t][:, h0:h0 + 512],
                                 start=True, stop=False)
                nc.tensor.matmul(pout[:, h0:h0 + 512], lhsT=rn2_ts[t][:],
                                 rhs=vones_t[:, h0:h0 + 512],
                                 start=False, stop=True)
            o_t = opool.tile([128, DIM], BF16, tag="o")
            if t % 2 == 0:
                nc.vector.tensor_copy(o_t[:], pout[:])
            else:
                nc.scalar.copy(o_t[:], pout[:])
            if use_g:
                nc.vector.tensor_tensor(o_t[:], o_t[:], gb_t[:], op=AluOpType.mult)
            if use_b:
                nc.vector.tensor_tensor(o_t[:], o_t[:], bb_t[:], op=AluOpType.add)
            nc.sync.dma_start(out_d[t * 128:(t + 1) * 128, :], o_t[:])

    nc.compile()
    return nc


def _get_program(flags):
    if flags not in _cache:
        _cache[flags] = _build(flags)
    return _cache[flags]


def _pairize(w):
    """[DIM, C] -> [128, DIM/128 * C] DoubleRow pair layout:
    out[p, P, i, c] = w[128*(2P+i)+p, c]."""
    c = w.shape[1]
    return np.ascontiguousarray(
        w.reshape(NP, 2, 128, c).transpose(2, 0, 1, 3).reshape(128, NJ * c))


def _make_in_maps(inputs):
    f32 = lambda a: np.ascontiguousarray(np.asarray(a, np.float32))
    bf = ml_dtypes.bfloat16
    fp8 = ml_dtypes.float8_e4m3
    x = f32(inputs["x"])
    xnb = f32(inputs["x_neighbor"])
    mask = np.ascontiguousarray(np.asarray(inputs["mask"], np.int32))
    nmask = f32(inputs["neighbor_mask"])
    qw, qb = f32(inputs["qw"]), f32(inputs["qb"])
    kw, kb = f32(inputs["kw"]), f32(inputs["kb"])
    vw, vb = f32(inputs["vw"]), f32(inputs["vb"])
    ow, ob = f32(inputs["ow"]), f32(inputs["ob"])
    ln_g, ln_b = f32(inputs["ln_g"]), f32(inputs["ln_b"])

    flags = (bool(qb.any()), bool(kb.any()), bool(vb.any()), bool(ob.any()),
             bool((ln_g != 1.0).any()), bool(ln_b.any()), bool((mask == 0).any()))
    use_qb, use_kb, use_vb, use_ob, use_g, use_b, use_mask = flags

    qwt8 = _pairize(qw.T * WS).astype(fp8)
    kw8 = _pairize(kw * WS).astype(fp8)
    vwt8 = _pairize(vw.T * WS).astype(fp8)
    owt8 = _pairize(ow.T * WS).astype(fp8)
    i128 = np.eye(128, dtype=bf)

    in_maps = []
    for b in range(BS):
        m = {
            "x": x[b].astype(bf),
            "xtp": _pairize(np.ascontiguousarray(x[b].T)).astype(fp8),
            "qwt": qwt8, "kw": kw8, "vwt": vwt8, "owt": owt8,
            "xnb": xnb[b].astype(bf),
            "nmc": nmask[b].astype(bf),
            "nmr": np.ascontiguousarray(nmask[b]),
            "i128": i128,
        }
        if use_qb:
            m["qb"] = qb
        if use_kb:
            m["kbt"] = np.ascontiguousarray(kb.reshape(NJ, 128).T).astype(bf)
        if use_vb:
            m["vbt"] = np.ascontiguousarray(
                (vb * (SPN * WS)).reshape(NJ, 128).T).astype(bf)
        if use_ob:
            m["ob"] = ob
        if use_g:
            m["lng"] = ln_g
        if use_b:
            m["lnb"] = ln_b
        if use_mask:
            m["mask"] = np.ascontiguousarray(mask[b])
        in_maps.append(m)
    return flags, in_maps


def kernel(**inputs):
    flags, in_maps = _make_in_maps(inputs)
    nc = _get_program(flags)
    res = bass_utils.run_bass_kernel_spmd(nc, in_maps, core_ids=list(range(N_CORES)))
    return np.stack([np.asarray(res.results[b]["out"]).astype(np.float32)
                     for b in range(BS)])


# revision 14
# speedup vs baseline: 1.1519x; 1.1519x over previous
"""Trainium2 Bass kernel for nn_NeighborAttention.

Algebraic structure: the attention query is one mean-pooled neighbor
vector per batch broadcast over the sequence, so the [seq, seq]
attention collapses to one weight vector per head and the attention
output is a single vector v per batch added to every row of x before
the LayerNorm.  k/v projections are never materialized.

Per core (data-parallel over batch): x bf16 (2 MB) + x^T fp8 pair-
layout (1 MB) + 4 weight matrices fp8 pair-layout scaled x64 (4 MB) +
bf16 output (2 MB).  All large GEMMs run in fp8 DoubleRow mode (two
128-row contraction chunks per instruction at 0.5 cyc/row).  The
softmax weights are scaled x256 via an exp bias so they sit in fp8
normal range; the scale cancels through the softmax denominator.
Residual+LayerNorm is evaluated ON THE PE as
  out_tile = diag(rstd) @ x_tile + [rstd; nmr]^T @ [v; ones]
with per-row x statistics from bn_stats during the DMA fill and the
v cross-term via one DoubleRow GEMM against x^T.  The scalar engine
runs only Exp/Sqrt/Square/Copy (resident ACT table sets).
Host-side prep is layout/dtype only (transpose, pair-interleave,
scale-by-constant, cast).
"""

import numpy as np
import ml_dtypes
from contextlib import ExitStack

try:
    import concourse.bass as bass
except ImportError:  # pragma: no cover - fallback for bare containers
    import sys
    sys.path.insert(0, "/opt/trn_rl_repo")
    import concourse.bass as bass

import concourse.tile as tile
from concourse import bacc, mybir
from concourse import bass_utils
from concourse.alu_op_type import AluOpType

F32 = mybir.dt.float32
BF16 = mybir.dt.bfloat16
FP8 = mybir.dt.float8e4
I32 = mybir.dt.int32
AF = mybir.ActivationFunctionType
AX = mybir.AxisListType
DR = mybir.MatmulPerfMode.DoubleRow

BS, SEQ, DIM, NH, DH, NNB = 8, 1024, 1024, 16, 64, 50
NT = SEQ // 128   # seq tiles
NJ = DIM // 128   # dim chunks
NP = NJ // 2      # contraction pairs
N_CORES = 8

WS = 64.0         # fp8 weight pre-scale (host)
SQB = 64.0        # q-vector scale inside blk (fp8 range)
SQK = 128.0       # qk scale (fp8 range); exp descales
LN_SW = 4.1588830833596715  # ln(64): softmax weight scale, cancels in rden
SPN = 16.0        # pooled scale (fp8 range)
SV = 64.0         # v scale for the Sxv cross-term operand

_cache = {}


def _build(flags):
    use_qb, use_kb, use_vb, use_ob, use_g, use_b, use_mask = flags
    nc = bacc.Bacc("TRN2", target_bir_lowering=False, debug=False,
                   enable_asserts=True, num_devices=N_CORES)

    def din(name, shape, dt):
        return nc.dram_tensor(name, shape, dt, kind="ExternalInput").ap()

    x_d = din("x", [SEQ, DIM], BF16)
    xtp_d = din("xtp", [128, NJ * SEQ], FP8)     # pair layout of x^T
    qwt_d = din("qwt", [128, NJ * DIM], FP8)     # pair layouts, x WS
    kw_d = din("kw", [128, NJ * DIM], FP8)
    vwt_d = din("vwt", [128, NJ * DIM], FP8)
    owt_d = din("owt", [128, NJ * DIM], FP8)
    xnb_d = din("xnb", [NNB, DIM], BF16)
    nmc_d = din("nmc", [NNB], BF16)
    nmr_d = din("nmr", [NNB], F32)
    i128_d = din("i128", [128, 128], BF16)
    qb_d = din("qb", [DIM], F32) if use_qb else None
    kbt_d = din("kbt", [128, NJ], BF16) if use_kb else None
    vb_d = din("vbt", [128, NJ], BF16) if use_vb else None  # x (SPN*WS)
    ob_d = din("ob", [DIM], F32) if use_ob else None
    g_d = din("lng", [DIM], F32) if use_g else None
    b_d = din("lnb", [DIM], F32) if use_b else None
    mask_d = din("mask", [SEQ], I32) if use_mask else None
    out_d = nc.dram_tensor("out", [SEQ, DIM], BF16, kind="ExternalOutput").ap()

    with tile.TileContext(nc) as tc, ExitStack() as ctx:
        wpool = ctx.enter_context(tc.tile_pool(name="wts", bufs=1))
        spool = ctx.enter_context(tc.tile_pool(name="small", bufs=1))
        opool = ctx.enter_context(tc.tile_pool(name="o", bufs=4))
        pwide = ctx.enter_context(tc.tile_pool(name="pw", bufs=2, space="PSUM"))
        psmall = ctx.enter_context(tc.tile_pool(name="ps", bufs=2, space="PSUM"))

        # ---------------- DMAs (issue order ~ arrival order) ----------------
        xnb_t = spool.tile([NNB, DIM], BF16, tag="xnb")
        nc.sync.dma_start(xnb_t[:], xnb_d[:])
        nmc_t = spool.tile([NNB, 1], BF16, tag="nmc")
        nc.sync.dma_start(nmc_t[:], nmc_d.unsqueeze(1))
        nmr_t = spool.tile([1, NNB], F32, tag="nmr")
        nc.sync.dma_start(nmr_t[:], nmr_d.unsqueeze(0))
        i128_t = spool.tile([128, 128], BF16, tag="i128")
        nc.sync.dma_start(i128_t[:], i128_d[:])

        def row_tile(d_ap, tag):
            t = spool.tile([1, DIM], F32, tag=tag)
            nc.sync.dma_start(t[:], d_ap.unsqueeze(0))
            return t

        qb_t = row_tile(qb_d, "qbr") if use_qb else None
        ob_t = row_tile(ob_d, "obr") if use_ob else None
        if use_vb:
            vbt_t = spool.tile([128, NJ], BF16, tag="vbt")
            nc.sync.dma_start(vbt_t[:], vb_d[:])
        g_t = row_tile(g_d, "gr") if use_g else None
        b_t = row_tile(b_d, "br") if use_b else None
        if use_kb:
            kbt_t = spool.tile([128, NJ], BF16, tag="kbt")
            nc.sync.dma_start(kbt_t[:], kbt_d[:])
        if use_mask:
            mrow_t = spool.tile([1, SEQ], I32, tag="mrow")
            nc.sync.dma_start(mrow_t[:], mask_d.unsqueeze(0))

        def load_pairs(d_ap, tagp, cols):
            """[128, NJ*cols] fp8 pair-layout matrix, two half-DMAs.
            Returns 4D view [128, NP, 2, cols]."""
            t = wpool.tile([128, NJ * cols], FP8, tag=tagp)
            half = (NJ * cols) // 2
            nc.sync.dma_start(t[:, 0:half], d_ap[:, 0:half])
            nc.sync.dma_start(t[:, half:], d_ap[:, half:])
            return t[:].rearrange("p (P i c) -> p P i c", P=NP, i=2)

        qwtp = load_pairs(qwt_d, "qwt", DIM)
        kwp = load_pairs(kw_d, "kw", DIM)
        xtp = load_pairs(xtp_d, "xt", SEQ)

        # x bf16 (residual + pooled source), as one [128, NT*DIM] tile
        xbig = wpool.tile([128, NT * DIM], BF16, tag="x")
        xv_d = x_d.rearrange("(t p) c -> p t c", p=128)
        xv = xbig[:].rearrange("p (t c) -> p t c", t=NT)
        nc.sync.dma_start(xv[:, 0:NT // 2, :], xv_d[:, 0:NT // 2, :])
        nc.sync.dma_start(xv[:, NT // 2:, :], xv_d[:, NT // 2:, :])
        x_t = [xbig[:, t * DIM:(t + 1) * DIM] for t in range(NT)]

        vwtp = load_pairs(vwt_d, "vwt", DIM)
        owtp = load_pairs(owt_d, "owt", DIM)

        ones11 = spool.tile([1, 1], BF16, tag="ones11")
        nc.vector.memset(ones11[:], 1.0)
        ones1x128 = spool.tile([1, 128], F32, tag="ones1x128")
        nc.vector.memset(ones1x128[:], 1.0)
        lnsw_t = spool.tile([NH, 1], F32, tag="lnsw")
        nc.vector.memset(lnsw_t[:], LN_SW)

        # Load the two ACT table sets (exp_and_others, sqrt_and_others)
        # during the DMA fill; only Exp/Sqrt/Square/Copy are used after.
        dummy_t = spool.tile([1, 1], F32, tag="dummy")
        nc.vector.memset(dummy_t[:], 1.0)
        nc.scalar.activation(dummy_t[:], dummy_t[:], AF.Exp)
        nc.scalar.activation(dummy_t[:], dummy_t[:], AF.Sqrt)

        # per-row bn stats of x during the fill (for the LN fixup)
        mvx_t = spool.tile([128, 2 * NT], F32, tag="mvx")
        mvxv = mvx_t[:].rearrange("p (t a) -> p t a", a=2)
        for t in range(NT):
            hv = x_t[t].rearrange("p (g f) -> p g f", g=2)
            st_t = spool.tile([128, 2, 6], F32, tag="st")
            nc.vector.bn_stats(st_t[:, 0, :], hv[:, 0, :])
            nc.vector.bn_stats(st_t[:, 1, :], hv[:, 1, :])
            nc.vector.bn_aggr(mvxv[:, t, :], st_t[:])

        def bcast_row(row_ap, out_tile, nrows):
            """out[p, :] = row[0, :] for p in range(nrows), via PE rank-1."""
            n = out_tile.shape[-1]
            pb = pwide.tile([128, DIM], F32, tag="wide")
            for h0 in range(0, n, 512):
                hi = min(h0 + 512, n)
                nc.tensor.matmul(pb[:nrows, h0:hi], lhsT=ones1x128[0:1, 0:nrows],
                                 rhs=row_ap[0:1, h0:hi], start=True, stop=True)
            nc.vector.tensor_copy(out_tile[:nrows, :], pb[:nrows, 0:n])

        def drmm(psum_ap, lhs4, rhs4, np_pairs):
            """DoubleRow GEMM: accumulate over np_pairs pair-chunks, two
            512-col halves.  lhs4: [128, np_pairs, 2, M] fp8 pair view."""
            for p in range(np_pairs):
                for h0 in (0, 512):
                    nc.tensor.matmul(psum_ap[:, h0:h0 + 512],
                                     lhsT=lhs4[:, p, :, :],
                                     rhs=rhs4[:, p, :, h0:h0 + 512],
                                     start=(p == 0), stop=(p == np_pairs - 1),
                                     perf_mode=DR)

        # zero-padded fp8 thin-operand pair tiles (M=16, col 0 real)
        def padded16(tag):
            t = spool.tile([128, NJ * 16], FP8, tag=tag)
            nc.vector.memset(t[:], 0.0)
            return t, t[:].rearrange("p (P i m) -> p P i m", P=NP, i=2)

        # ---------------- neighbor pooling: SxnT -> fp8 pad tile -----------
        ps8w = pwide.tile([128, DIM], F32, tag="wide")
        ps8 = ps8w[:, 0:16]
        for j in range(NJ):
            nc.tensor.matmul(ps8[:, j:j + 1], lhsT=xnb_t[:, j * 128:(j + 1) * 128],
                             rhs=nmc_t[:], start=True, stop=True)
        sxnt_t, sxntv = padded16("sxnt")
        sxcols = sxnt_t[:].rearrange("p (c m) -> p c m", m=16)[:, :, 0]
        nc.vector.tensor_copy(sxcols[:, 0:NJ], ps8[:, 0:NJ])
        cnt_t = spool.tile([1, 1], F32, tag="cnt")
        nc.vector.reduce_sum(cnt_t[:], nmr_t[:], AX.X)
        rcnt_t = spool.tile([1, 1], F32, tag="rcnt")
        nc.vector.reciprocal(rcnt_t[:], cnt_t[:])

        # ------- qvec = (qw @ xn + qb) / 8;  pqv is x WS -------------------
        pqv = pwide.tile([128, DIM], F32, tag="wide")
        drmm(pqv[0:16, :], sxntv, qwtp, NP)
        qvec_t = spool.tile([1, DIM], F32, tag="qvec")
        nc.vector.tensor_scalar(qvec_t[:], pqv[0:1, :], rcnt_t[:], 0.125 / WS,
                                AluOpType.mult, AluOpType.mult)
        if use_qb:
            qb8_t = spool.tile([1, DIM], F32, tag="qb8")
            nc.vector.tensor_scalar_mul(qb8_t[:], qb_t[:], 0.125)
            nc.vector.tensor_tensor(qvec_t[:], qvec_t[:], qb8_t[:], op=AluOpType.add)

        # ------- head-blocked qvec, fp8 pair tiles (x SQB) -----------------
        qvr_t = spool.tile([1, DIM], BF16, tag="rowb")
        nc.vector.tensor_scalar_mul(qvr_t[:], qvec_t[:], SQB)
        blkp_t = []
        for P in range(NP):
            bt = spool.tile([128, 32], FP8, tag=f"blk{P}")
            nc.vector.memset(bt[:], 0.0)
            for i in (0, 1):
                j = 2 * P + i
                pt = psmall.tile([128, 16], BF16, tag="psmb")
                nc.tensor.transpose(pt[:, 0:1], qvr_t[0:1, j * 128:(j + 1) * 128],
                                    ones11[:])
                nc.vector.tensor_copy(bt[0:64, 16 * i + 2 * j:16 * i + 2 * j + 1],
                                      pt[0:64, 0:1])
                nc.vector.tensor_copy(bt[64:128, 16 * i + 2 * j + 1:16 * i + 2 * j + 2],
                                      pt[64:128, 0:1])
            blkp_t.append(bt[:].rearrange("p (i m) -> p i m", i=2))

        # ------- qk[h, c] (x SQB*WS in psum -> qk_t x SQK) -----------------
        pqk = pwide.tile([128, DIM], F32, tag="wide")
        for P in range(NP):
            for h0 in (0, 512):
                nc.tensor.matmul(pqk[0:NH, h0:h0 + 512], lhsT=blkp_t[P],
                                 rhs=kwp[:, P, :, h0:h0 + 512],
                                 start=(P == 0), stop=(P == NP - 1),
                                 perf_mode=DR)
        qk_t = spool.tile([NH, DIM], BF16, tag="qk")
        nc.scalar.mul(qk_t[:], pqk[0:NH, :], SQK / (SQB * WS))
        if use_kb:
            pqkbw = pwide.tile([128, DIM], F32, tag="wide")
            pqkb = pqkbw[:, 0:16]
            for P in range(NP):
                for i in (0, 1):
                    j = 2 * P + i
                    nc.tensor.matmul(pqkb[0:NH, 0:1],
                                     lhsT=blkp_t[P][:, i, :],
                                     rhs=kbt_t[:, j:j + 1],
                                     start=(j == 0), stop=(j == NJ - 1))
            qkb_t = spool.tile([NH, 1], F32, tag="qkb")
            # kb path: pqkb is x SQB; exp bias wants true qkb / SQK ... the
            # exp computes exp(psc/SQK + bias), so bias = qkb (true scale).
            nc.vector.tensor_scalar_mul(qkb_t[:], pqkb[0:NH, 0:1], 1.0 / SQB)

        # ------- scoresT [NH, SEQ] = qk @ x^T (x SQK) ----------------------
        qktp_t = []
        for P in range(NP):
            t = spool.tile([128, 32], FP8, tag=f"qkt{P}")
            for i in (0, 1):
                j = 2 * P + i
                pt = psmall.tile([128, 16], BF16, tag="psmb")
                nc.tensor.transpose(pt[:], qk_t[:, j * 128:(j + 1) * 128],
                                    i128_t[0:NH, 0:NH])
                nc.vector.tensor_copy(t[:, 16 * i:16 * i + 16], pt[:])
            qktp_t.append(t[:].rearrange("p (i m) -> p i m", i=2))
        psc = pwide.tile([128, DIM], F32, tag="wide")
        for P in range(NP):
            for h0 in (0, 512):
                nc.tensor.matmul(psc[0:NH, h0:h0 + 512], lhsT=qktp_t[P],
                                 rhs=xtp[:, P, :, h0:h0 + 512],
                                 start=(P == 0), stop=(P == NP - 1),
                                 perf_mode=DR)

        # ------- softmax over keys; w is x 256 via exp bias (cancels) ------
        w_t = spool.tile([NH, SEQ], BF16, tag="w")
        den_t = spool.tile([NH, 1], F32, tag="den")
        if use_kb:
            expb_t = spool.tile([NH, 1], F32, tag="expb")
            nc.vector.tensor_scalar(expb_t[:], qkb_t[:], LN_SW, None,
                                    AluOpType.add)
            expbias = expb_t[:]
        else:
            expbias = lnsw_t[:]
        if not use_mask:
            nc.scalar.activation(w_t[:], psc[0:NH, :], AF.Exp, bias=expbias,
                                 scale=1.0 / SQK, accum_out=den_t[:])
        else:
            nc.scalar.activation(w_t[:], psc[0:NH, :], AF.Exp, bias=expbias,
                                 scale=1.0 / SQK)
            mrowf_t = spool.tile([1, SEQ], F32, tag="mrowf")
            nc.vector.tensor_copy(mrowf_t[:], mrow_t[:])
            ind_t = spool.tile([1, SEQ], F32, tag="ind")
            nc.vector.tensor_scalar(ind_t[:], mrowf_t[:], 0.0, None,
                                    AluOpType.not_equal)
            m16_t = spool.tile([NH, SEQ], F32, tag="m16")
            bcast_row(ind_t, m16_t, NH)
            nc.vector.scalar_tensor_tensor(w_t[:], w_t[:], 1.0, m16_t[:],
                                           AluOpType.mult, AluOpType.mult,
                                           accum_out=den_t[:])
        rden_t = spool.tile([NH, 1], F32, tag="rden")
        nc.vector.reciprocal(rden_t[:], den_t[:])

        # ------- pooled[h, c] = sum_s w x (w x256 cancels via rden) --------
        wt_t = []
        for j in range(NT):
            t = spool.tile([128, NH], BF16, tag=f"wt{j}")
            pt = psmall.tile([128, 16], BF16, tag="psmb")
            nc.tensor.transpose(pt[:], w_t[:, j * 128:(j + 1) * 128],
                                i128_t[0:NH, 0:NH])
            nc.vector.tensor_copy(t[:], pt[:])
            wt_t.append(t)
        ppl = pwide.tile([128, DIM], F32, tag="wide")
        for j in range(NT):
            for h0 in (0, 512):
                nc.tensor.matmul(ppl[0:NH, h0:h0 + 512], lhsT=wt_t[j][:],
                                 rhs=x_t[j][:, h0:h0 + 512],
                                 start=(j == 0), stop=(j == NT - 1))
        pn_t = spool.tile([NH, DIM], BF16, tag="pn")
        nc.vector.tensor_scalar(pn_t[:], ppl[0:NH, :], rden_t[:], SPN,
                                AluOpType.mult, AluOpType.mult)

        # ------- context diag blocks (psum x SPN*WS) -----------------------
        pntp_t = []
        for P in range(NP):
            t = spool.tile([128, 32], FP8, tag=f"pnt{P}")
            for i in (0, 1):
                j = 2 * P + i
                pt = psmall.tile([128, 16], BF16, tag="psmb")
                nc.tensor.transpose(pt[:], pn_t[:, j * 128:(j + 1) * 128],
                                    i128_t[0:NH, 0:NH])
                nc.vector.tensor_copy(t[:, 16 * i:16 * i + 16], pt[:])
            pntp_t.append(t[:].rearrange("p (i m) -> p i m", i=2))
        pcx = pwide.tile([128, DIM], F32, tag="wide")
        for P in range(NP):
            for h0 in (0, 512):
                nc.tensor.matmul(pcx[0:NH, h0:h0 + 512], lhsT=pntp_t[P],
                                 rhs=vwtp[:, P, :, h0:h0 + 512],
                                 start=(P == 0), stop=(P == NP - 1),
                                 perf_mode=DR)
        # ctx[o] = pcx[head(o), o]: per-chunk transpose, pick halves into
        # the zero-padded fp8 pair tile (col 0 of each 16-block).
        pcs_t = spool.tile([NH, DIM], BF16, tag="pcs")
        nc.scalar.copy(pcs_t[:], pcx[0:NH, :])
        cxt_t, cxtv = padded16("cxt")
        cxcols = cxt_t[:].rearrange("p (c m) -> p c m", m=16)[:, :, 0]
        for j in range(NJ):
            pt = psmall.tile([128, 16], BF16, tag="psmb")
            nc.tensor.transpose(pt[:], pcs_t[:, j * 128:(j + 1) * 128],
                                i128_t[0:NH, 0:NH])
            nc.vector.tensor_copy(cxcols[0:64, j:j + 1], pt[0:64, 2 * j:2 * j + 1])
            nc.vector.tensor_copy(cxcols[64:128, j:j + 1],
                                  pt[64:128, 2 * j + 1:2 * j + 2])
        if use_vb:
            nc.vector.tensor_tensor(cxcols[:, 0:NJ], cxcols[:, 0:NJ], vbt_t[:],
                                    op=AluOpType.add)

        # ------- out_vec = ow @ ctx + ob (psum x SPN*WS*WS) ----------------
        pov = pwide.tile([128, DIM], F32, tag="wide")
        drmm(pov[0:16, :], cxtv, owtp, NP)
        bvec_t = spool.tile([1, DIM], F32, tag="bvec")
        nc.vector.tensor_scalar_mul(bvec_t[:], pov[0:1, :], 1.0 / (SPN * WS * WS))
        if use_ob:
            nc.vector.tensor_tensor(bvec_t[:], bvec_t[:], ob_t[:], op=AluOpType.add)

        # =================== residual + LayerNorm ==========================
        # stats: mu_h = mu_x + mu_v ; var_h = var_x + var_v
        #        + 2*(Sxv/D - mu_x*mu_v) with Sxv = x @ v via PE on x^T.
        # output: out = diag(rstd) @ x + [rstd; nmr]^T @ [v; ones]  (PE)

        # v stats
        sv_t = spool.tile([1, 2], F32, tag="sv")
        nc.vector.reduce_sum(sv_t[0:1, 0:1], bvec_t[:], AX.X)
        junk_t = spool.tile([1, DIM], F32, tag="junk")
        nc.scalar.activation(junk_t[:], bvec_t[:], AF.Square,
                             accum_out=sv_t[0:1, 1:2])
        nc.vector.tensor_scalar_mul(sv_t[:], sv_t[:], 1.0 / DIM)
        muv2_t = spool.tile([1, 1], F32, tag="muv2")
        nc.vector.tensor_tensor(muv2_t[:], sv_t[0:1, 0:1], sv_t[0:1, 0:1],
                                op=AluOpType.mult)
        nc.vector.tensor_tensor(sv_t[0:1, 1:2], sv_t[0:1, 1:2], muv2_t[:],
                                op=AluOpType.subtract)
        bsc_t = spool.tile([128, 2], F32, tag="bsc")
        nc.gpsimd.partition_broadcast(bsc_t[:], sv_t[:])

        # Sxv via DoubleRow on x^T: operand v x SV in fp8 pad tile
        bvr_t = spool.tile([1, DIM], BF16, tag="bvr")
        nc.vector.tensor_scalar_mul(bvr_t[:], bvec_t[:], SV)
        bvt_t, bvtv = padded16("bvt")
        bvcols = bvt_t[:].rearrange("p (c m) -> p c m", m=16)[:, :, 0]
        for j in range(NJ):
            pt = psmall.tile([128, 16], BF16, tag="psmb")
            nc.tensor.transpose(pt[:, 0:1], bvr_t[0:1, j * 128:(j + 1) * 128],
                                ones11[:])
            nc.vector.tensor_copy(bvcols[:, j:j + 1], pt[:, 0:1])
        psxv = pwide.tile([128, DIM], F32, tag="wide")
        drmm(psxv[0:16, :], bvtv, xtp, NP)
        sxvr_t = spool.tile([1, SEQ], BF16, tag="sxvr")
        nc.vector.tensor_copy(sxvr_t[:], psxv[0:1, :])
        sxvc_t = spool.tile([128, NT], F32, tag="sxvc")
        for t in range(NT):
            pt = psmall.tile([128, 16], BF16, tag="psmb")
            nc.tensor.transpose(pt[:, 0:1], sxvr_t[0:1, t * 128:(t + 1) * 128],
                                ones11[:])
            nc.vector.tensor_copy(sxvc_t[:, t:t + 1], pt[:, 0:1])

        # batched fixup on [128, NT]
        a_t = spool.tile([128, NT], F32, tag="fa")
        nc.vector.tensor_scalar_mul(a_t[:], sxvc_t[:], 2.0 / (SV * DIM))
        b2_t = spool.tile([128, NT], F32, tag="fb")
        nc.vector.tensor_scalar_mul(b2_t[:], mvxv[:, :, 0], bsc_t[:, 0:1])
        c_t = spool.tile([128, NT], F32, tag="fc")
        nc.vector.scalar_tensor_tensor(c_t[:], b2_t[:], -2.0, a_t[:],
                                       AluOpType.mult, AluOpType.add)
        d_t = spool.tile([128, NT], F32, tag="fd")
        nc.vector.tensor_scalar(d_t[:], c_t[:], bsc_t[:, 1:2], None,
                                AluOpType.add)
        e_t = spool.tile([128, NT], F32, tag="fe")
        nc.vector.tensor_tensor(e_t[:], d_t[:], mvxv[:, :, 1], op=AluOpType.add)
        srt_t = spool.tile([128, NT], F32, tag="srt")
        nc.scalar.activation(srt_t[:], e_t[:], AF.Sqrt, bias=0.0)
        rstd_t = spool.tile([128, NT], F32, tag="rstd")
        nc.vector.reciprocal(rstd_t[:], srt_t[:])
        muh_t = spool.tile([128, NT], F32, tag="muh")
        nc.vector.tensor_scalar(muh_t[:], mvxv[:, :, 0], bsc_t[:, 0:1], None,
                                AluOpType.add)
        rn_t = spool.tile([128, 2 * NT], BF16, tag="rn")
        rnv = rn_t[:].rearrange("p (t a) -> p t a", a=2)
        nc.vector.tensor_copy(rnv[:, :, 0], rstd_t[:])
        nc.vector.scalar_tensor_tensor(rnv[:, :, 1], muh_t[:], -1.0, rstd_t[:],
                                       AluOpType.mult, AluOpType.mult)

        # [v; ones] moving operand and per-tile [rstd; nmr] rows
        vones_t = spool.tile([2, DIM], BF16, tag="vones")
        nc.vector.memset(vones_t[:], 1.0)
        nc.vector.tensor_copy(vones_t[0:1, :], bvec_t[:])
        rn2_ts = []
        for t in range(NT):
            pt = psmall.tile([128, 128], BF16, tag="psmc")
            nc.tensor.transpose(pt[0:2, :], rnv[:, t, :], i128_t[:, :])
            r2 = spool.tile([2, 128], BF16, tag=f"rn2_{t}")
            nc.vector.tensor_copy(r2[:], pt[0:2, :])
            rn2_ts.append(r2)

        if use_g:
            gb_t = spool.tile([128, DIM], F32, tag="gb")
            bcast_row(g_t, gb_t, 128)
        if use_b:
            bb_t = spool.tile([128, DIM], F32, tag="bb")
            bcast_row(b_t, bb_t, 128)

        # per-tile: PE affine -> psum -> copy (V/S alternate) -> DMA
        diag_ts = []
        for t in range(NT):
            diag_t = spool.tile([128, 128], BF16, tag=f"diag{t}")
            nc.vector.tensor_scalar_mul(diag_t[:], i128_t[:], rstd_t[:, t:t + 1])
            diag_ts.append(diag_t)
        for t in range(NT):
            pout = pwide.tile([128, DIM], F32, tag="wide")
            for h0 in (0, 512):
                nc.tensor.matmul(pout[:, h0:h0 + 512], lhsT=diag_ts[t][:],
                                 rhs=x_t[t][:, h0:h0 + 512],
                                 start=True, stop=False)
                nc.tensor.matmul(pout[:, h0:h0 + 512], lhsT=rn2_ts[t][:],
                                 rhs=vones_t[:, h0:h0 + 512],
                                 start=False, stop=True)
            o_t = opool.tile([128, DIM], BF16, tag="o")
            if t % 2 == 0:
                nc.vector.tensor_copy(o_t[:], pout[:])
            else:
                nc.scalar.copy(o_t[:], pout[:])
            if use_g:
                nc.vector.tensor_tensor(o_t[:], o_t[:], gb_t[:], op=AluOpType.mult)
            if use_b:
                nc.vector.tensor_tensor(o_t[:], o_t[:], bb_t[:], op=AluOpType.add)
            nc.sync.dma_start(out_d[t * 128:(t + 1) * 128, :], o_t[:])

    nc.compile()
    return nc


def _get_program(flags):
    if flags not in _cache:
        _cache[flags] = _build(flags)
    return _cache[flags]


def _pairize(w):
    """[DIM, C] -> [128, DIM/128 * C] DoubleRow pair layout:
    out[p, P, i, c] = w[128*(2P+i)+p, c]."""
    c = w.shape[1]
    return np.ascontiguousarray(
        w.reshape(NP, 2, 128, c).transpose(2, 0, 1, 3).reshape(128, NJ * c))


def _make_in_maps(inputs):
    f32 = lambda a: np.ascontiguousarray(np.asarray(a, np.float32))
    bf = ml_dtypes.bfloat16
    fp8 = ml_dtypes.float8_e4m3
    x = f32(inputs["x"])
    xnb = f32(inputs["x_neighbor"])
    mask = np.ascontiguousarray(np.asarray(inputs["mask"], np.int32))
    nmask = f32(inputs["neighbor_mask"])
    qw, qb = f32(inputs["qw"]), f32(inputs["qb"])
    kw, kb = f32(inputs["kw"]), f32(inputs["kb"])
    vw, vb = f32(inputs["vw"]), f32(inputs["vb"])
    ow, ob = f32(inputs["ow"]), f32(inputs["ob"])
    ln_g, ln_b = f32(inputs["ln_g"]), f32(inputs["ln_b"])

    flags = (bool(qb.any()), bool(kb.any()), bool(vb.any()), bool(ob.any()),
             bool((ln_g != 1.0).any()), bool(ln_b.any()), bool((mask == 0).any()))
    use_qb, use_kb, use_vb, use_ob, use_g, use_b, use_mask = flags

    qwt8 = _pairize(qw.T * WS).astype(fp8)
    kw8 = _pairize(kw * WS).astype(fp8)
    vwt8 = _pairize(vw.T * WS).astype(fp8)
    owt8 = _pairize(ow.T * WS).astype(fp8)
    i128 = np.eye(128, dtype=bf)

    in_maps = []
    for b in range(BS):
        m = {
            "x": x[b].astype(bf),
            "xtp": _pairize(np.ascontiguousarray(x[b].T)).astype(fp8),
            "qwt": qwt8, "kw": kw8, "vwt": vwt8, "owt": owt8,
            "xnb": xnb[b].astype(bf),
            "nmc": nmask[b].astype(bf),
            "nmr": np.ascontiguousarray(nmask[b]),
            "i128": i128,
        }
        if use_qb:
            m["qb"] = qb
        if use_kb:
            m["kbt"] = np.ascontiguousarray(kb.reshape(NJ, 128).T).astype(bf)
        if use_vb:
            m["vbt"] = np.ascontiguousarray(
                (vb * (SPN * WS)).reshape(NJ, 128).T).astype(bf)
        if use_ob:
            m["ob"] = ob
        if use_g:
            m["lng"] = ln_g
        if use_b:
            m["lnb"] = ln_b
        if use_mask:
            m["mask"] = np.ascontiguousarray(mask[b])
        in_maps.append(m)
    return flags, in_maps


def kernel(**inputs):
    flags, in_maps = _make_in_maps(inputs)
    nc = _get_program(flags)
    res = bass_utils.run_bass_kernel_spmd(nc, in_maps, core_ids=list(range(N_CORES)))
    return np.stack([np.asarray(res.results[b]["out"]).astype(np.float32)
                     for b in range(BS)])
